# revision 2
# baseline (speedup 1.0000x reference)
"""Masked dense layer  out = tanh(x @ (w*mask_w) + b*mask_b)  on 8 TRN2 cores.

Data-parallel: x is sharded along the batch axis (32768 rows per core);
w/b/mask_w/mask_b are replicated. The baseline (f32 HWDGE slabs + one DVE
AFFINE_MUL_REDUCE per row) was co-paced by DVE: AMR is a 1x-mode custom op
(~604ns per 512-elem row), so DVE busy (~155us) matched DMA busy and added
a ~9us tail after the last slab landed.

v2 splits the work so DMA is the only pacer:
- x slabs stream via the gpsimd SWDGE queue with f32->bf16 cast-in-flight
  (HBM reads stay f32 - mandatory traffic - but SBUF writes halve and DVE
  gets 16-bit operands).
- Per 4-row slab: DVE does rows 0-1 with AMR directly (1x, 604ns each) and
  one 2x_1p-packed bf16 tensor_mul for rows 2-3 (~594ns); ACT reduces the
  two product rows via activation(Copy, accum_out=...) (~613ns each) and
  applies Tanh(+bias) per chunk. DVE ~1.8us/slab, ACT ~1.6us/slab, both
  under the ~2.4us/slab DMA pace.
- Params load on the otherwise-idle HWDGE sync ring; output is written in
  quarters during the stream so the tail is only the last quarter.
"""

import numpy as np

import concourse.bacc as bacc
import concourse.bass as bass
import concourse.tile as tile
from concourse import mybir
from concourse.bass_utils import run_bass_kernel_spmd

N, F = 262144, 512
C = 8                 # cores
R = N // C            # rows per core  = 32768
P = 128               # SBUF partitions
RP = R // P           # rows per partition = 256
T = 4                 # rows-per-partition per DMA slab (1 MiB f32 per dma_start)
NCHUNK = RP // T      # 64 slabs per core
QUARTER = NCHUNK // 4

_cached_nc = None


def build_bass() -> bass.Bass:
    nc = bacc.Bacc()

    x = nc.declare_dram_parameter("x", [R, F], mybir.dt.float32, isOutput=False)
    w = nc.declare_dram_parameter("w", [F, 1], mybir.dt.float32, isOutput=False)
    b = nc.declare_dram_parameter("b", [1], mybir.dt.float32, isOutput=False)
    mask_w = nc.declare_dram_parameter(
        "mask_w", [F, 1], mybir.dt.int32, isOutput=False
    )
    mask_b = nc.declare_dram_parameter("mask_b", [1], mybir.dt.int32, isOutput=False)
    out = nc.declare_dram_parameter("out", [R, 1], mybir.dt.float32, isOutput=True)

    # partition p <- rows [p*RP, (p+1)*RP); per partition each slab is a
    # contiguous T*F*4 = 8 KiB DRAM run.
    x_r = x[:, :].rearrange("(p r) f -> p r f", p=P)      # [128, 256, 512]
    out_r = out[:, :].rearrange("(p r) one -> p (r one)", p=P)  # [128, 256]

    def bcast(src_handle, count):
        """DRAM AP replicating a contiguous `count`-element vector across P partitions."""
        ap = src_handle[:]
        return bass.AP(tensor=ap.tensor, offset=ap.offset, ap=[[0, P], [1, count]])

    def rep_mid(ap2d, k):
        """View a [P, F] SBUF AP as [P, k, F] with 0-stride middle dim."""
        return bass.AP(
            tensor=ap2d.tensor,
            offset=ap2d.offset,
            ap=[ap2d.ap[0], [0, k], ap2d.ap[1]],
        )

    with tile.TileContext(nc) as tc:
        with (
            tc.tile_pool(name="singles", bufs=1) as singles,
            tc.tile_pool(name="slabs", bufs=16) as slabs,
            tc.tile_pool(name="prods", bufs=3) as prods,
            tc.tile_pool(name="vjunk", bufs=2) as vjunk,
            tc.tile_pool(name="ajunk", bufs=2) as ajunk,
            tc.tile_pool(name="stages", bufs=4) as stages,
        ):
            # masked weights, broadcast to all partitions: wm[p, f] = w[f]*mask_w[f]
            # (HWDGE sync-ring loads; DVE casts the int32 mask.)
            wb = singles.tile([P, F], mybir.dt.float32)
            nc.sync.dma_start(out=wb, in_=bcast(w, F))
            mwi = singles.tile([P, F], mybir.dt.int32)
            nc.sync.dma_start(out=mwi, in_=bcast(mask_w, F))
            mw = singles.tile([P, F], mybir.dt.float32)
            nc.vector.tensor_copy(mw, mwi)  # i32 -> f32
            wm = singles.tile([P, F], mybir.dt.float32)
            nc.vector.tensor_mul(wm, wb, mw)
            wm_bf = singles.tile([P, F], mybir.dt.bfloat16)
            nc.vector.tensor_copy(wm_bf, wm)  # f32 -> bf16

            # masked bias, per-partition scalar: bm[p, 0] = b[0]*mask_b[0]
            bb = singles.tile([P, 1], mybir.dt.float32)
            nc.sync.dma_start(out=bb, in_=bcast(b, 1))
            mbi = singles.tile([P, 1], mybir.dt.int32)
            nc.sync.dma_start(out=mbi, in_=bcast(mask_b, 1))
            mb = singles.tile([P, 1], mybir.dt.float32)
            nc.vector.tensor_copy(mb, mbi)  # i32 -> f32
            bm = singles.tile([P, 1], mybir.dt.float32)
            nc.vector.tensor_mul(bm, bb, mb)

            outt = singles.tile([P, RP], mybir.dt.float32)
            for c in range(NCHUNK):
                slab = slabs.tile([P, T, F], mybir.dt.bfloat16, tag="slab")
                nc.gpsimd.dma_start(out=slab, in_=x_r[:, c * T : (c + 1) * T, :])
                stage = stages.tile([P, T], mybir.dt.float32, tag="stage")
                # rows 0-1: fused mul+reduce on DVE
                for t in range(2):
                    junk = vjunk.tile([P, F], mybir.dt.bfloat16, tag="vj")
                    nc.vector.affine_mul_reduce(
                        out=junk,
                        accum_out=stage[:, t : t + 1],
                        in0=slab[:, t, :],
                        in1=wm_bf,
                        scale=1.0,
                        bias=0.0,
                    )
                # rows 2-3: 2x-packed bf16 multiply on DVE, reduce on ACT
                prod = prods.tile([P, 2, F], mybir.dt.bfloat16, tag="prod")
                nc.vector.tensor_mul(prod, slab[:, 2:4, :], rep_mid(wm_bf[:, :], 2))
                for t in range(2):
                    aj = ajunk.tile([P, F], mybir.dt.bfloat16, tag="aj")
                    nc.scalar.activation(
                        out=aj,
                        in_=prod[:, t, :],
                        func=mybir.ActivationFunctionType.Copy,
                        accum_out=stage[:, 2 + t : 3 + t],
                    )
                nc.scalar.activation(
                    out=outt[:, c * T : (c + 1) * T],
                    in_=stage,
                    func=mybir.ActivationFunctionType.Tanh,
                    bias=bm,
                    scale=1.0,
                )
                # stream the finished quarters out on the idle sync ring
                if (c + 1) % QUARTER == 0 and c + 1 < NCHUNK:
                    q0 = (c + 1 - QUARTER) * T
                    q1 = (c + 1) * T
                    nc.sync.dma_start(
                        out=out_r[:, q0:q1], in_=outt[:, q0:q1]
                    )
            q0 = (NCHUNK - QUARTER) * T
            nc.sync.dma_start(out=out_r[:, q0:], in_=outt[:, q0:])

    nc.finalize()
    return nc


def run_sharded(inputs: dict, **run_kwargs):
    """Shard inputs, run on 8 cores, gather. Returns (output, BassKernelResults)."""
    global _cached_nc
    if _cached_nc is None:
        _cached_nc = build_bass()
    nc = _cached_nc

    x = np.ascontiguousarray(np.asarray(inputs["x"], dtype=np.float32))
    w = np.ascontiguousarray(np.asarray(inputs["w"], dtype=np.float32))
    b = np.ascontiguousarray(np.asarray(inputs["b"], dtype=np.float32))
    mask_w = np.ascontiguousarray(np.asarray(inputs["mask_w"], dtype=np.int32))
    mask_b = np.ascontiguousarray(np.asarray(inputs["mask_b"], dtype=np.int32))

    in_maps = [
        {
            "x": x[i * R : (i + 1) * R],
            "w": w,
            "b": b,
            "mask_w": mask_w,
            "mask_b": mask_b,
        }
        for i in range(C)
    ]
    res = run_bass_kernel_spmd(nc, in_maps, core_ids=list(range(C)), **run_kwargs)
    outs = [res.results[i]["out"] for i in range(C)]
    return np.concatenate(outs, axis=0), res


def kernel(x, w, b, mask_w, mask_b) -> np.ndarray:
    out, _ = run_sharded(
        {"x": x, "w": w, "b": b, "mask_w": mask_w, "mask_b": mask_b}
    )
    return out
